# revision 30
# baseline (speedup 1.0000x reference)
"""TRN2 Bass kernel for nn_BrushModel (5-attr x 16-expert 3-layer MLP stack).

Strategy: data-parallel over N (16384 rows -> 2048/core across 8 cores),
expert weights replicated on every core. Per attr, two phases:
  Phase A (per expert): L1 z1 = [W1|b1] @ [x;1] as K=d+1 row-tiled f32r
    matmuls (n-chunks in different PE row groups + psum banks), evacuated
    as h1 = relu(z1) by ACT/DVE; then L2 z2 = W2 @ h1 (K=128, f32r),
    evacuated as h2 = relu(z2 + b2) (fp16).
  Phase B (per group of 4 experts): L3 z3 = W3 @ h2 as col-tiled fp16
    matmuls (4 experts in 4 PE col groups sharing psum tiles), evacuated
    with + b3, DMA'd out per-expert.
All psum tiles come from one 4-slot pool of [128,1024] (8 banks total) so
the PE can run ahead and keep the HAM clock warm.
Host side: shard/replicate inputs, gather and transpose outputs.
"""

import numpy as np

import concourse.bass as bass
import concourse.tile as tile
from concourse import bacc, mybir
from concourse.bass import ts
from concourse.bass_utils import run_bass_kernel_spmd

P = 16
N_TOTAL = 16384
H = 128
N_CORES = 8
NPC = N_TOTAL // N_CORES  # 2048
ATTRS = [("means", 3), ("scales", 3), ("rgbs", 3), ("quats", 4), ("opacities", 1)]

f32 = mybir.dt.float32
f16 = mybir.dt.float16
MM_DT = f16                # L1/L2 matmul dtype
L3_DT = f16                # L3 matmul dtype (2-byte, required for col tiling)

CHUNK = 512            # moving free dim per matmul
N_CHUNKS = NPC // CHUNK  # 4
PS_W = 1024            # psum tile width (2 banks); 4 slots = all 8 banks
LDW_OPT = False        # bacc standalone Ldweights incompatible with walrus ldw-opt
FILLERS = False        # dummy PE warmers measured net-negative


def _patch_ldw_opt():
    from concourse import bass_utils

    if getattr(bass_utils, "_ldw_patched", False):
        return
    orig = bass_utils.run_command

    def patched(argv, **kwargs):
        argv = [
            a.replace("--enable-ldw-opt=false", "--enable-ldw-opt=true")
            for a in argv
        ]
        return orig(argv, **kwargs)

    bass_utils.run_command = patched
    bass_utils._ldw_patched = True


class _EvacBalancer:
    """Greedy-assign evacuation ops to ACT or DVE by accumulated time."""

    def __init__(self, nc):
        self.nc = nc
        self.act_ns = 0.0
        self.dve_ns = 0.0

    def emit(self, out_ap, in_ap, width, bias=None, relu=False, force=None):
        act_cost = (width + 172) / 1.2 + 32
        dve_cost = (width + 120) / 0.96 + 45
        if force is not None:
            use_act = force == "act"
        else:
            use_act = self.act_ns + act_cost <= self.dve_ns + dve_cost
        nc = self.nc
        if use_act:
            self.act_ns += act_cost
            func = (
                mybir.ActivationFunctionType.Relu
                if relu
                else mybir.ActivationFunctionType.Identity
            )
            nc.scalar.activation(
                out_ap, in_ap, func, bias=bias if bias is not None else 0.0
            )
        else:
            self.dve_ns += dve_cost
            if relu:
                if bias is not None:
                    nc.vector.tensor_scalar(
                        out_ap, in_ap, bias, 0.0,
                        mybir.AluOpType.add, mybir.AluOpType.max,
                    )
                else:
                    nc.vector.tensor_scalar(
                        out_ap, in_ap, 0.0, None, mybir.AluOpType.max
                    )
            else:
                if bias is not None:
                    nc.vector.tensor_scalar(
                        out_ap, in_ap, bias, None, mybir.AluOpType.add
                    )
                else:
                    nc.vector.tensor_copy(out_ap, in_ap)


def _build_program():
    if LDW_OPT:
        _patch_ldw_opt()
    nc = bacc.Bacc(
        "TRN2", target_bir_lowering=False, debug=False, num_devices=N_CORES
    )

    # --- DRAM parameters ---
    x_dram, w1_dram, w2_dram, w3_dram, b2_dram, b3e_dram, out_dram = (
        {}, {}, {}, {}, {}, {}, {},
    )
    for a, (name, d) in enumerate(ATTRS):
        k1 = d + 1
        x_dram[a] = nc.dram_tensor(f"x_{name}", [k1, NPC], MM_DT, kind="ExternalInput").ap()
        w1_dram[a] = nc.dram_tensor(f"w1_{name}", [k1, P, H], MM_DT, kind="ExternalInput").ap()
        w2_dram[a] = nc.dram_tensor(f"w2_{name}", [H, P, H], MM_DT, kind="ExternalInput").ap()
        w3_dram[a] = nc.dram_tensor(f"w3_{name}", [H, P, 32], L3_DT, kind="ExternalInput").ap()
        b2_dram[a] = nc.dram_tensor(f"b2_{name}", [H, P], f32, kind="ExternalInput").ap()
        b3e_dram[a] = nc.dram_tensor(f"b3e_{name}", [H, P], f32, kind="ExternalInput").ap()
        out_dram[a] = nc.dram_tensor(f"out_{name}", [P, H, CHUNK], f32, kind="ExternalOutput").ap()

    with tile.TileContext(nc) as tc:
        with (
            tc.tile_pool(name="consts", bufs=1) as cpool,
            tc.tile_pool(name="xpool", bufs=2) as xpool,
            tc.tile_pool(name="h1", bufs=6) as h1_pool,
            tc.tile_pool(name="h2", bufs=6) as h2_pool,
            tc.tile_pool(name="osb", bufs=6) as osb_pool,
            tc.tile_pool(name="psum", bufs=4, space="PSUM") as ps_pool,
        ):
            x_tiles = {}

            def load_x(a):
                name, d = ATTRS[a]
                k1 = d + 1
                xt = xpool.tile([128, NPC], MM_DT, tag="x", name=f"x_{name}_sb")
                for j in range(4):
                    nc.sync.dma_start(
                        out=xt[32 * j : 32 * j + k1, :], in_=x_dram[a][:]
                    )
                x_tiles[a] = xt

            # --- preload weights ---
            w1_sb, w2_sb, w3_sb, b2_sb, b3e_sb = {}, {}, {}, {}, {}
            for a, (name, d) in enumerate(ATTRS):
                k1 = d + 1
                w1_sb[a] = cpool.tile([128, P, H], MM_DT, tag=f"w1_{name}", name=f"w1_{name}")
                for j in range(4):
                    nc.sync.dma_start(
                        out=w1_sb[a][32 * j : 32 * j + k1, :, :], in_=w1_dram[a][:]
                    )
                w2_sb[a] = cpool.tile([H, P, H], MM_DT, tag=f"w2_{name}", name=f"w2_{name}")
                nc.sync.dma_start(out=w2_sb[a][:], in_=w2_dram[a][:])
                w3_sb[a] = cpool.tile([H, P, 32], L3_DT, tag=f"w3_{name}", name=f"w3_{name}")
                nc.sync.dma_start(out=w3_sb[a][:], in_=w3_dram[a][:])
                b2_sb[a] = cpool.tile([H, P], f32, tag=f"b2_{name}", name=f"b2_{name}")
                nc.sync.dma_start(out=b2_sb[a][:], in_=b2_dram[a][:])
                b3e_sb[a] = cpool.tile([H, P], f32, tag=f"b3e_{name}", name=f"b3e_{name}")
                nc.sync.dma_start(out=b3e_sb[a][:], in_=b3e_dram[a][:])
                if a < 2:
                    load_x(a)

            bal = _EvacBalancer(nc)

            for a, (name, d) in enumerate(ATTRS):
                k1 = d + 1
                if a not in x_tiles:
                    load_x(a)
                x_sb = x_tiles.pop(a)
                # prefetch next attr's x a whole attr ahead
                if a + 1 < len(ATTRS) and (a + 1) not in x_tiles:
                    load_x(a + 1)

                # ---- per-expert chains, two experts interleaved so the
                # PE's in-order stream always has independent ready work ----
                h1_tiles = {}
                h2_tiles = {}

                def emit_l1(e, a=a, k1=k1, x_sb=x_sb):
                    h1 = h1_pool.tile([128, NPC], MM_DT, tag="h1", name="h1")
                    h1_tiles[e] = h1
                    for half in range(NPC // PS_W):
                        psA = ps_pool.tile([128, PS_W], f32, tag="ps", name="psA")
                        for cc in range(PS_W // CHUNK):
                            c = half * (PS_W // CHUNK) + cc
                            nc.tensor.matmul(
                                psA[:, ts(cc, CHUNK)],
                                w1_sb[a][32 * c : 32 * c + k1, e, :],
                                x_sb[32 * c : 32 * c + k1, ts(c, CHUNK)],
                                start=True,
                                stop=True,
                                tile_position=(32 * c, 0),
                            )
                        bal.emit(h1[:, ts(half, PS_W)], psA[:], PS_W, relu=True)

                def emit_l2(e, a=a):
                    h1 = h1_tiles.pop(e)
                    h2 = h2_pool.tile([128, NPC], L3_DT, tag="h2", name="h2")
                    h2_tiles[e] = h2
                    for half in range(NPC // PS_W):
                        psB = ps_pool.tile([128, PS_W], f32, tag="ps", name="psB")
                        for cc in range(PS_W // CHUNK):
                            c = half * (PS_W // CHUNK) + cc
                            nc.tensor.matmul(
                                psB[:, ts(cc, CHUNK)],
                                w2_sb[a][:, e, :],
                                h1[:, ts(c, CHUNK)],
                                start=True,
                                stop=True,
                            )
                        bal.emit(
                            h2[:, ts(half, PS_W)],
                            psB[:],
                            PS_W,
                            bias=b2_sb[a][:, e : e + 1],
                            relu=True,
                        )

                def emit_l3(e, a=a, d=d):
                    # all 4 n-chunks of this expert packed into ONE psum
                    # bank: chunk c lands in col-group strip c.
                    h2 = h2_tiles.pop(e)
                    psC = ps_pool.tile([128, CHUNK], f32, tag="ps", name="psC")
                    for c in range(N_CHUNKS):
                        nc.tensor.matmul(
                            psC[32 * c : 32 * c + 32, :],
                            w3_sb[a][:, e, :],
                            h2[:, ts(c, CHUNK)],
                            start=True,
                            stop=True,
                            tile_position=(0, 32 * c),
                        )
                    osb = osb_pool.tile([128, CHUNK], f32, tag="osb", name="osb")
                    bal.emit(osb[:], psC[:], CHUNK, bias=b3e_sb[a][:, e : e + 1])
                    # single DMA of the whole tile; host unpacks the strips
                    nc.sync.dma_start(out=out_dram[a][e], in_=osb[:])

                for ee in range(0, P, 2):
                    emit_l1(ee)
                    emit_l1(ee + 1)
                    emit_l2(ee)
                    emit_l2(ee + 1)
                    emit_l3(ee)
                    emit_l3(ee + 1)

    nc.compile()
    return nc


_PROG = None


def _get_program():
    global _PROG
    if _PROG is None:
        _PROG = _build_program()
    return _PROG


def _prepare_core_inputs(inputs):
    """Host-side: build per-core input maps."""
    core_maps = [{} for _ in range(N_CORES)]
    for a, (name, d) in enumerate(ATTRS):
        k1 = d + 1
        x = inputs[name]  # [N_TOTAL, d]
        W1 = inputs[f"{name}_W1"]  # [P, H, d]
        b1 = inputs[f"{name}_b1"]  # [P, H]
        W2 = inputs[f"{name}_W2"]  # [P, H, H]
        b2 = inputs[f"{name}_b2"]  # [P, H]
        W3 = inputs[f"{name}_W3"]  # [P, d, H]
        b3 = inputs[f"{name}_b3"]  # [P, d]

        # w1 lhsT: [k1, P, H];  w1[k, e, h] = W1[e, h, k], w1[d, e, h] = b1[e, h]
        w1 = np.empty((k1, P, H), np.float16)
        w1[:d] = W1.transpose(2, 0, 1).astype(np.float16)
        w1[d] = b1.astype(np.float16)
        # w2 lhsT: [H, P, H]; w2[k, e, m] = W2[e, m, k]
        w2 = np.ascontiguousarray(W2.transpose(2, 0, 1).astype(np.float16))
        # w3 lhsT: [H, P, 32]; w3[k, e, o] = W3[e, o, k], zero-padded to M=32
        # so every col-tiled matmul writes its full 32-partition psum strip
        w3 = np.zeros((H, P, 32), np.float16)
        w3[:, :, :d] = W3.transpose(2, 0, 1).astype(np.float16)
        # b2 : [H, P]
        b2t = np.ascontiguousarray(b2.T)
        # b3e : [128, P]; b3e[32*c + o, e] = b3[e, o] (chunk c = col strip)
        b3e = np.zeros((H, P), np.float32)
        for c in range(4):
            b3e[32 * c : 32 * c + d, :] = b3.T

        for core in range(N_CORES):
            xa = np.empty((k1, NPC), np.float16)
            xa[:d] = x[core * NPC : (core + 1) * NPC].T
            xa[d] = 1.0
            m = core_maps[core]
            m[f"x_{name}"] = np.ascontiguousarray(xa)
            m[f"w1_{name}"] = w1
            m[f"w2_{name}"] = w2
            m[f"w3_{name}"] = w3
            m[f"b2_{name}"] = b2t
            m[f"b3e_{name}"] = b3e
    return core_maps


def _assemble_outputs(results):
    """results: list per core of {out_<name>: [P, d, NPC]} -> tuple of 5 arrays."""
    outs = []
    for a, (name, d) in enumerate(ATTRS):
        full = np.empty((P, d, N_TOTAL), np.float32)
        for core in range(N_CORES):
            stage = results[core][f"out_{name}"]  # [P, 128, CHUNK]
            # strip (32c+o, n) holds z3[e, o, 512c+n]
            z = stage.reshape(P, 4, 32, CHUNK)[:, :, :d, :].transpose(0, 2, 1, 3)
            full[:, :, core * NPC : (core + 1) * NPC] = z.reshape(P, d, NPC)
        outs.append(np.ascontiguousarray(full.transpose(0, 2, 1).reshape(P * N_TOTAL, d)))
    return tuple(outs)


def kernel(**inputs):
    nc = _get_program()
    core_maps = _prepare_core_inputs(inputs)
    res = run_bass_kernel_spmd(nc, core_maps, list(range(N_CORES)))
    return _assemble_outputs(res.results)


# revision 31
# speedup vs baseline: 1.1137x; 1.1137x over previous
"""TRN2 Bass kernel for nn_BrushModel (5-attr x 16-expert 3-layer MLP stack).

Strategy: data-parallel over N (16384 rows -> 2048/core across 8 cores),
expert weights replicated on every core. Per attr, two phases:
  Phase A (per expert): L1 z1 = [W1|b1] @ [x;1] as K=d+1 row-tiled f32r
    matmuls (n-chunks in different PE row groups + psum banks), evacuated
    as h1 = relu(z1) by ACT/DVE; then L2 z2 = W2 @ h1 (K=128, f32r),
    evacuated as h2 = relu(z2 + b2) (fp16).
  Phase B (per group of 4 experts): L3 z3 = W3 @ h2 as col-tiled fp16
    matmuls (4 experts in 4 PE col groups sharing psum tiles), evacuated
    with + b3, DMA'd out per-expert.
All psum tiles come from one 4-slot pool of [128,1024] (8 banks total) so
the PE can run ahead and keep the HAM clock warm.
Host side: shard/replicate inputs, gather and transpose outputs.
"""

import numpy as np

import concourse.bass as bass
import concourse.tile as tile
from concourse import bacc, mybir
from concourse.bass import ts
from concourse.bass_utils import run_bass_kernel_spmd

P = 16
N_TOTAL = 16384
H = 128
N_CORES = 8
NPC = N_TOTAL // N_CORES  # 2048
ATTRS = [("means", 3), ("scales", 3), ("rgbs", 3), ("quats", 4), ("opacities", 1)]

f32 = mybir.dt.float32
f16 = mybir.dt.float16
MM_DT = f16                # L1/L2 matmul dtype
L3_DT = f16                # L3 matmul dtype (2-byte, required for col tiling)

CHUNK = 512            # moving free dim per matmul
N_CHUNKS = NPC // CHUNK  # 4
PS_W = 1024            # psum tile width (2 banks); 4 slots = all 8 banks
LDW_OPT = False        # bacc standalone Ldweights incompatible with walrus ldw-opt
FILLERS = False        # dummy PE warmers measured net-negative


def _patch_ldw_opt():
    from concourse import bass_utils

    if getattr(bass_utils, "_ldw_patched", False):
        return
    orig = bass_utils.run_command

    def patched(argv, **kwargs):
        argv = [
            a.replace("--enable-ldw-opt=false", "--enable-ldw-opt=true")
            for a in argv
        ]
        return orig(argv, **kwargs)

    bass_utils.run_command = patched
    bass_utils._ldw_patched = True


class _EvacBalancer:
    """Greedy-assign evacuation ops to ACT or DVE by accumulated time."""

    def __init__(self, nc):
        self.nc = nc
        self.act_ns = 0.0
        self.dve_ns = 0.0

    def emit(self, out_ap, in_ap, width, bias=None, relu=False, force=None):
        act_cost = (width + 172) / 1.2 + 32
        dve_cost = (width + 120) / 0.96 + 45
        if force is not None:
            use_act = force == "act"
        else:
            use_act = self.act_ns + act_cost <= self.dve_ns + dve_cost
        nc = self.nc
        if use_act:
            self.act_ns += act_cost
            func = (
                mybir.ActivationFunctionType.Relu
                if relu
                else mybir.ActivationFunctionType.Identity
            )
            nc.scalar.activation(
                out_ap, in_ap, func, bias=bias if bias is not None else 0.0
            )
        else:
            self.dve_ns += dve_cost
            if relu:
                if bias is not None:
                    nc.vector.tensor_scalar(
                        out_ap, in_ap, bias, 0.0,
                        mybir.AluOpType.add, mybir.AluOpType.max,
                    )
                else:
                    nc.vector.tensor_scalar(
                        out_ap, in_ap, 0.0, None, mybir.AluOpType.max
                    )
            else:
                if bias is not None:
                    nc.vector.tensor_scalar(
                        out_ap, in_ap, bias, None, mybir.AluOpType.add
                    )
                else:
                    nc.vector.tensor_copy(out_ap, in_ap)


def _build_program():
    if LDW_OPT:
        _patch_ldw_opt()
    nc = bacc.Bacc(
        "TRN2", target_bir_lowering=False, debug=False, num_devices=N_CORES
    )

    # --- DRAM parameters ---
    x_dram, w1_dram, w2_dram, w3_dram, b2_dram, b3e_dram, out_dram = (
        {}, {}, {}, {}, {}, {}, {},
    )
    for a, (name, d) in enumerate(ATTRS):
        k1 = d + 1
        x_dram[a] = nc.dram_tensor(f"x_{name}", [k1, NPC], MM_DT, kind="ExternalInput").ap()
        w1_dram[a] = nc.dram_tensor(f"w1_{name}", [k1, P, H], MM_DT, kind="ExternalInput").ap()
        w2_dram[a] = nc.dram_tensor(f"w2_{name}", [H, P, H], MM_DT, kind="ExternalInput").ap()
        w3_dram[a] = nc.dram_tensor(f"w3_{name}", [H, P, 32], L3_DT, kind="ExternalInput").ap()
        b2_dram[a] = nc.dram_tensor(f"b2_{name}", [H, P], f32, kind="ExternalInput").ap()
        b3e_dram[a] = nc.dram_tensor(f"b3e_{name}", [H, P], f32, kind="ExternalInput").ap()
        out_dram[a] = nc.dram_tensor(f"out_{name}", [P, H, CHUNK], f32, kind="ExternalOutput").ap()

    with tile.TileContext(nc) as tc:
        with (
            tc.tile_pool(name="consts", bufs=1) as cpool,
            tc.tile_pool(name="xpool", bufs=2) as xpool,
            tc.tile_pool(name="h1", bufs=6) as h1_pool,
            tc.tile_pool(name="h2", bufs=6) as h2_pool,
            tc.tile_pool(name="osb", bufs=6) as osb_pool,
            tc.tile_pool(name="psum", bufs=4, space="PSUM") as ps_pool,
        ):
            x_tiles = {}

            def load_x(a):
                name, d = ATTRS[a]
                k1 = d + 1
                xt = xpool.tile([128, NPC], MM_DT, tag="x", name=f"x_{name}_sb")
                for j in range(4):
                    nc.sync.dma_start(
                        out=xt[32 * j : 32 * j + k1, :], in_=x_dram[a][:]
                    )
                x_tiles[a] = xt

            # --- preload weights ---
            w1_sb, w2_sb, w3_sb, b2_sb, b3e_sb = {}, {}, {}, {}, {}
            for a, (name, d) in enumerate(ATTRS):
                k1 = d + 1
                w1_sb[a] = cpool.tile([128, P, H], MM_DT, tag=f"w1_{name}", name=f"w1_{name}")
                for j in range(4):
                    nc.sync.dma_start(
                        out=w1_sb[a][32 * j : 32 * j + k1, :, :], in_=w1_dram[a][:]
                    )
                w2_sb[a] = cpool.tile([H, P, H], MM_DT, tag=f"w2_{name}", name=f"w2_{name}")
                nc.sync.dma_start(out=w2_sb[a][:], in_=w2_dram[a][:])
                w3_sb[a] = cpool.tile([H, P, 32], L3_DT, tag=f"w3_{name}", name=f"w3_{name}")
                nc.sync.dma_start(out=w3_sb[a][:], in_=w3_dram[a][:])
                b2_sb[a] = cpool.tile([H, P], f32, tag=f"b2_{name}", name=f"b2_{name}")
                nc.sync.dma_start(out=b2_sb[a][:], in_=b2_dram[a][:])
                b3e_sb[a] = cpool.tile([H, P], f32, tag=f"b3e_{name}", name=f"b3e_{name}")
                nc.sync.dma_start(out=b3e_sb[a][:], in_=b3e_dram[a][:])
                if a < 2:
                    load_x(a)

            bal = _EvacBalancer(nc)

            for a, (name, d) in enumerate(ATTRS):
                k1 = d + 1
                if a not in x_tiles:
                    load_x(a)
                x_sb = x_tiles.pop(a)
                # prefetch next attr's x a whole attr ahead
                if a + 1 < len(ATTRS) and (a + 1) not in x_tiles:
                    load_x(a + 1)

                # ---- per-expert chains, two experts interleaved so the
                # PE's in-order stream always has independent ready work ----
                h1_tiles = {}
                h2_tiles = {}

                def emit_l1(e, a=a, k1=k1, x_sb=x_sb):
                    h1 = h1_pool.tile([128, NPC], MM_DT, tag="h1", name="h1")
                    h1_tiles[e] = h1
                    for half in range(NPC // PS_W):
                        psA = ps_pool.tile([128, PS_W], f32, tag="ps", name="psA")
                        for cc in range(PS_W // CHUNK):
                            c = half * (PS_W // CHUNK) + cc
                            nc.tensor.matmul(
                                psA[:, ts(cc, CHUNK)],
                                w1_sb[a][32 * c : 32 * c + k1, e, :],
                                x_sb[32 * c : 32 * c + k1, ts(c, CHUNK)],
                                start=True,
                                stop=True,
                                tile_position=(32 * c, 0),
                            )
                        bal.emit(h1[:, ts(half, PS_W)], psA[:], PS_W, relu=True)

                def emit_l2(e, a=a):
                    h1 = h1_tiles.pop(e)
                    h2 = h2_pool.tile([128, NPC], L3_DT, tag="h2", name="h2")
                    h2_tiles[e] = h2
                    for half in range(NPC // PS_W):
                        psB = ps_pool.tile([128, PS_W], f32, tag="ps", name="psB")
                        for cc in range(PS_W // CHUNK):
                            c = half * (PS_W // CHUNK) + cc
                            nc.tensor.matmul(
                                psB[:, ts(cc, CHUNK)],
                                w2_sb[a][:, e, :],
                                h1[:, ts(c, CHUNK)],
                                start=True,
                                stop=True,
                            )
                        bal.emit(
                            h2[:, ts(half, PS_W)],
                            psB[:],
                            PS_W,
                            bias=b2_sb[a][:, e : e + 1],
                            relu=True,
                        )

                def emit_l3(e, a=a, d=d):
                    # all 4 n-chunks of this expert packed into ONE psum
                    # bank: chunk c lands in col-group strip c.
                    h2 = h2_tiles.pop(e)
                    psC = ps_pool.tile([128, CHUNK], f32, tag="ps", name="psC")
                    for c in range(N_CHUNKS):
                        nc.tensor.matmul(
                            psC[32 * c : 32 * c + 32, :],
                            w3_sb[a][:, e, :],
                            h2[:, ts(c, CHUNK)],
                            start=True,
                            stop=True,
                            tile_position=(0, 32 * c),
                        )
                    osb = osb_pool.tile([128, CHUNK], f32, tag="osb", name="osb")
                    bal.emit(osb[:], psC[:], CHUNK, bias=b3e_sb[a][:, e : e + 1])
                    # single DMA of the whole tile; host unpacks the strips
                    nc.sync.dma_start(out=out_dram[a][e], in_=osb[:])

                pending_l3 = []
                for ee in range(0, P, 2):
                    emit_l1(ee)
                    emit_l1(ee + 1)
                    emit_l2(ee)
                    emit_l2(ee + 1)
                    if pending_l3:
                        emit_l3(pending_l3[0])
                        emit_l3(pending_l3[1])
                    pending_l3 = [ee, ee + 1]
                emit_l3(pending_l3[0])
                emit_l3(pending_l3[1])

    nc.compile()
    return nc


_PROG = None


def _get_program():
    global _PROG
    if _PROG is None:
        _PROG = _build_program()
    return _PROG


def _prepare_core_inputs(inputs):
    """Host-side: build per-core input maps."""
    core_maps = [{} for _ in range(N_CORES)]
    for a, (name, d) in enumerate(ATTRS):
        k1 = d + 1
        x = inputs[name]  # [N_TOTAL, d]
        W1 = inputs[f"{name}_W1"]  # [P, H, d]
        b1 = inputs[f"{name}_b1"]  # [P, H]
        W2 = inputs[f"{name}_W2"]  # [P, H, H]
        b2 = inputs[f"{name}_b2"]  # [P, H]
        W3 = inputs[f"{name}_W3"]  # [P, d, H]
        b3 = inputs[f"{name}_b3"]  # [P, d]

        # w1 lhsT: [k1, P, H];  w1[k, e, h] = W1[e, h, k], w1[d, e, h] = b1[e, h]
        w1 = np.empty((k1, P, H), np.float16)
        w1[:d] = W1.transpose(2, 0, 1).astype(np.float16)
        w1[d] = b1.astype(np.float16)
        # w2 lhsT: [H, P, H]; w2[k, e, m] = W2[e, m, k]
        w2 = np.ascontiguousarray(W2.transpose(2, 0, 1).astype(np.float16))
        # w3 lhsT: [H, P, 32]; w3[k, e, o] = W3[e, o, k], zero-padded to M=32
        # so every col-tiled matmul writes its full 32-partition psum strip
        w3 = np.zeros((H, P, 32), np.float16)
        w3[:, :, :d] = W3.transpose(2, 0, 1).astype(np.float16)
        # b2 : [H, P]
        b2t = np.ascontiguousarray(b2.T)
        # b3e : [128, P]; b3e[32*c + o, e] = b3[e, o] (chunk c = col strip)
        b3e = np.zeros((H, P), np.float32)
        for c in range(4):
            b3e[32 * c : 32 * c + d, :] = b3.T

        for core in range(N_CORES):
            xa = np.empty((k1, NPC), np.float16)
            xa[:d] = x[core * NPC : (core + 1) * NPC].T
            xa[d] = 1.0
            m = core_maps[core]
            m[f"x_{name}"] = np.ascontiguousarray(xa)
            m[f"w1_{name}"] = w1
            m[f"w2_{name}"] = w2
            m[f"w3_{name}"] = w3
            m[f"b2_{name}"] = b2t
            m[f"b3e_{name}"] = b3e
    return core_maps


def _assemble_outputs(results):
    """results: list per core of {out_<name>: [P, d, NPC]} -> tuple of 5 arrays."""
    outs = []
    for a, (name, d) in enumerate(ATTRS):
        full = np.empty((P, d, N_TOTAL), np.float32)
        for core in range(N_CORES):
            stage = results[core][f"out_{name}"]  # [P, 128, CHUNK]
            # strip (32c+o, n) holds z3[e, o, 512c+n]
            z = stage.reshape(P, 4, 32, CHUNK)[:, :, :d, :].transpose(0, 2, 1, 3)
            full[:, :, core * NPC : (core + 1) * NPC] = z.reshape(P, d, NPC)
        outs.append(np.ascontiguousarray(full.transpose(0, 2, 1).reshape(P * N_TOTAL, d)))
    return tuple(outs)


def kernel(**inputs):
    nc = _get_program()
    core_maps = _prepare_core_inputs(inputs)
    res = run_bass_kernel_spmd(nc, core_maps, list(range(N_CORES)))
    return _assemble_outputs(res.results)
